# revision 1
# baseline (speedup 1.0000x reference)
"""Trainium2 Bass kernel for nn_DecoderBlockBVL (B=2,V=8,L=256,C=768,H=12).

Sharding: 8 cores; core c handles batch b=c//4, query-slice g=c%4
(rows [g*64,(g+1)*64) of every view). Phase 1 (per-view self-attn) is
computed redundantly for the whole batch on each core (phase 2 needs
k/v for all tokens); phase-2 queries and the MLP cover only the core's
512 tokens. The host permutes each view's rows so the core's slice
sits at the front -> every core runs one identical SPMD program.

Layouts: residual stream token-major [tok, C]; matmul operands
feature-major (x^T) via PE transpose after each LN; weights
pre-transposed on the host to [C_in, F]. Matmuls run in float32r
(full PE rate at moving dim >= 256); attention probs/V and the fc2
operands in bf16.
"""

import numpy as np
import ml_dtypes

import concourse.bass as bass
import concourse.bacc as bacc
import concourse.mybir as mybir
import concourse.tile as tile
from concourse.bass_utils import run_bass_kernel_spmd
from concourse.masks import make_identity

dt = mybir.dt
F32 = dt.float32
F32R = dt.float32r
BF16 = dt.bfloat16
AF = mybir.ActivationFunctionType
ALU = mybir.AluOpType

B, V, L, C, H = 2, 8, 256, 768, 12
HD = C // H          # 64
S = V * L            # 2048
HID = 3072
NCORES = 8
G = 4                # cores per batch
QS = L // G          # 64 queries per view per core
MYQ = V * QS         # 512 tokens per core
SCALE = HD ** -0.5
CK = C // 128        # 6
SK = S // 128        # 16
HK = HID // 128      # 24
NHALF = ((0, 384), (384, 384))


def _kr(v):
    """allowed key prefix length for query view v (block-causal mask)"""
    return 512 if v < 2 else 256 * (v + 1)


def _build(ln_identity: bool, zero_bias: bool, sim_gelu: bool = False):
    nc = bacc.Bacc()

    xb = nc.declare_dram_parameter("xb", [S, C], F32, isOutput=False)
    wqkv = nc.declare_dram_parameter("wqkv_t", [C, 3 * C], BF16, isOutput=False)
    wproj = nc.declare_dram_parameter("wproj_t", [C, C], BF16, isOutput=False)
    wq = nc.declare_dram_parameter("wq_t", [C, C], BF16, isOutput=False)
    wk = nc.declare_dram_parameter("wk_t", [C, C], BF16, isOutput=False)
    wv = nc.declare_dram_parameter("wv_t", [C, C], BF16, isOutput=False)
    wcp = nc.declare_dram_parameter("wcproj_t", [C, C], BF16, isOutput=False)
    wf1 = nc.declare_dram_parameter("wfc1_t", [C, HID], BF16, isOutput=False)
    wf2 = nc.declare_dram_parameter("wfc2_t", [HID, C], BF16, isOutput=False)
    out = nc.declare_dram_parameter("out", [MYQ, C], F32, isOutput=True)

    lng = lnb = bias = f1b = None
    if not ln_identity:
        lng = nc.declare_dram_parameter("ln_g", [3, C], F32, isOutput=False)
        lnb = nc.declare_dram_parameter("ln_b", [3, C], F32, isOutput=False)
    if not zero_bias:
        bias = nc.declare_dram_parameter("bias3", [3, C], F32, isOutput=False)
        f1b = nc.declare_dram_parameter("fc1_b", [HID], F32, isOutput=False)

    x1d = nc.dram_tensor("x1d", [S, C], F32)  # phase-1 output spill

    with tile.TileContext(nc) as tc, \
         tc.tile_pool(name="consts", bufs=1) as consts:
        identb = consts.tile([128, 128], BF16)
        make_identity(nc, identb)
        eps = consts.tile([128, 1], F32)
        nc.vector.memset(eps, 1e-5)

        gbt = bbt = bias_bc = f1b_t = None
        if not ln_identity:
            gbt = consts.tile([128, 3, C], F32)
            bbt = consts.tile([128, 3, C], F32)
            for t, src in ((gbt, lng), (bbt, lnb)):
                bc = bass.AP(tensor=src.tensor, offset=src.offset,
                             ap=[[0, 128]] + list(src.ap))
                nc.gpsimd.dma_start(out=t[:], in_=bc)
        if not zero_bias:
            bias_bc = consts.tile([128, 3, C], F32)
            bc = bass.AP(tensor=bias.tensor, offset=bias.offset,
                         ap=[[0, 128]] + list(bias.ap))
            nc.gpsimd.dma_start(out=bias_bc[:], in_=bc)
            f1b_t = consts.tile([128, HK], F32)
            nc.gpsimd.dma_start(out=f1b_t[:], in_=f1b.rearrange("(a p) -> p a", p=128))

        def ln(pool, x_ap, h_ap, which):
            """layernorm over free dim C; x_ap/h_ap [128, C]"""
            st = pool.tile([128, 3, 6], F32, tag="ln_st")
            for sg in range(3):
                nc.vector.bn_stats(out=st[:, sg, :],
                                   in_=x_ap[:, sg * 256:(sg + 1) * 256])
            mv = pool.tile([128, 2], F32, tag="ln_mv")
            nc.vector.bn_aggr(out=mv[:], in_=st[:])
            nm = pool.tile([128, 2], F32, tag="ln_nm")  # [neg-mean, rstd]
            nc.vector.tensor_scalar_mul(nm[:, 0:1], mv[:, 0:1], -1.0)
            nc.scalar.activation(nm[:, 1:2], mv[:, 1:2], AF.Sqrt, bias=eps[:])
            nc.vector.reciprocal(nm[:, 1:2], nm[:, 1:2])
            nc.vector.tensor_scalar(h_ap, x_ap, nm[:, 0:1], nm[:, 1:2],
                                    ALU.add, ALU.mult)
            if not ln_identity:
                nc.vector.tensor_mul(h_ap, h_ap, gbt[:, which, :])
                nc.vector.tensor_add(h_ap, h_ap, bbt[:, which, :])

        def transpose_cols(psp, src, dst, j, n):
            """n [128,128] bf16 blocks src(mc) -> dst[:, j, :] ([128, n*128])"""
            ps = psp.tile([128, n * 128], BF16, tag="scb")
            for mc in range(n):
                nc.tensor.matmul(ps[:, mc * 128:(mc + 1) * 128], src(mc),
                                 identb[:], is_transpose=True)
            nc.any.tensor_copy(dst[:, j, :], ps[:])

        # =================== phase 1: per-view self-attention ===================
        with tc.tile_pool(name="p1w", bufs=1) as p1w, \
             tc.tile_pool(name="p1b", bufs=2) as p1b, \
             tc.tile_pool(name="p1s", bufs=3) as p1s, \
             tc.tile_pool(name="ps_sc", bufs=2, space="PSUM") as ps_sc, \
             tc.tile_pool(name="ps_mb", bufs=2, space="PSUM") as ps_mb, \
             tc.tile_pool(name="ps_o", bufs=2, space="PSUM") as ps_o:

            wqkv_s = p1w.tile([128, CK, 3 * C], BF16)
            wproj_s = p1w.tile([128, CK, C], BF16)
            for kc in range(CK):
                nc.sync.dma_start(out=wqkv_s[:, kc, :], in_=wqkv[kc * 128:(kc + 1) * 128, :])
                nc.sync.dma_start(out=wproj_s[:, kc, :], in_=wproj[kc * 128:(kc + 1) * 128, :])

            for v in range(V):
                xv = p1b.tile([128, 2, C], F32, tag="xv")
                for mc in range(2):
                    nc.sync.dma_start(out=xv[:, mc, :],
                                      in_=xb[v * L + mc * 128: v * L + (mc + 1) * 128, :])
                h1 = p1b.tile([128, 2, C], BF16, tag="h1")
                for mc in range(2):
                    ln(p1s, xv[:, mc, :], h1[:, mc, :], 0)
                h1T = p1b.tile([128, CK, 256], BF16, tag="h1T")
                for j in range(CK):
                    transpose_cols(ps_sc, lambda mc: h1[:, mc, j * 128:(j + 1) * 128],
                                   h1T, j, 2)

                # q^T,k^T feature-major [1536, 256]
                qkT = p1b.tile([128, 12, 256], BF16, tag="qkT")
                for mo in range(12):
                    ps = ps_sc.tile([128, 256], F32, tag="sc")
                    for kc in range(CK):
                        nc.tensor.matmul(ps[:], wqkv_s[:, kc, mo * 128:(mo + 1) * 128],
                                         h1T[:, kc, :], start=kc == 0, stop=kc == CK - 1)
                    nc.any.tensor_copy(qkT[:, mo, :], ps[:])
                # v token-major bf16 [256, 768]
                v1 = p1b.tile([128, 2, C], BF16, tag="v1")
                for mt in range(2):
                    pss = [ps_mb.tile([128, 384], F32, tag="mb", name=f"mbh{i}") for i in range(2)]
                    for kc in range(CK):
                        for i, (no, nn_) in enumerate(NHALF):
                            nc.tensor.matmul(pss[i][:],
                                             h1T[:, kc, mt * 128:(mt + 1) * 128],
                                             wqkv_s[:, kc, 2 * C + no:2 * C + no + nn_],
                                             start=kc == 0, stop=kc == CK - 1)
                    for i, (no, nn_) in enumerate(NHALF):
                        nc.any.tensor_copy(v1[:, mt, no:no + nn_], pss[i][:])

                o1T = p1b.tile([128, CK, 256], BF16, tag="o1T")
                for hp in range(6):
                    ops = ps_o.tile([128, 256], F32, tag="o")
                    for hh in range(2):
                        h = hp * 2 + hh
                        qh = qkT[hh * 64:(hh + 1) * 64, hp, :]       # [64, 256]
                        kh = qkT[hh * 64:(hh + 1) * 64, 6 + hp, :]   # [64, 256]
                        sps = ps_sc.tile([128, 2, 256], F32, tag="sc")
                        for mc in range(2):
                            nc.tensor.matmul(sps[:, mc, :], qh[:, mc * 128:(mc + 1) * 128],
                                             kh, start=True, stop=True)
                        probs = p1s.tile([128, 2, 256], BF16, tag="probs")
                        sums = p1s.tile([128, 2], F32, tag="sums")
                        for mc in range(2):
                            nc.scalar.activation(probs[:, mc, :], sps[:, mc, :], AF.Exp,
                                                 scale=SCALE, accum_out=sums[:, mc:mc + 1])
                        nc.vector.reciprocal(sums[:], sums[:])
                        for mc in range(2):
                            nc.vector.tensor_scalar_mul(probs[:, mc, :], probs[:, mc, :],
                                                        sums[:, mc:mc + 1])
                        pTps = ps_sc.tile([128, 2, 256], BF16, tag="scb")
                        for kb in range(2):
                            for mc in range(2):
                                nc.tensor.matmul(pTps[:, kb, mc * 128:(mc + 1) * 128],
                                                 probs[:, mc, kb * 128:(kb + 1) * 128],
                                                 identb[:], is_transpose=True)
                        pT = p1s.tile([128, 2, 256], BF16, tag="pT")
                        for kb in range(2):
                            nc.any.tensor_copy(pT[:, kb, :], pTps[:, kb, :])
                        for kb in range(2):
                            nc.tensor.matmul(ops[hh * 64:(hh + 1) * 64, :],
                                             v1[:, kb, h * 64:(h + 1) * 64], pT[:, kb, :],
                                             start=kb == 0, stop=kb == 1)
                    nc.any.tensor_copy(o1T[:, hp, :], ops[:])

                # proj + residual -> x1 (token-major), spill to DRAM
                for mt in range(2):
                    pss = [ps_mb.tile([128, 384], F32, tag="mb", name=f"mbh{i}") for i in range(2)]
                    for kc in range(CK):
                        for i, (no, nn_) in enumerate(NHALF):
                            nc.tensor.matmul(pss[i][:],
                                             o1T[:, kc, mt * 128:(mt + 1) * 128],
                                             wproj_s[:, kc, no:no + nn_],
                                             start=kc == 0, stop=kc == CK - 1)
                    x1v = p1b.tile([128, C], F32, tag="x1v")
                    for i, (no, nn_) in enumerate(NHALF):
                        nc.vector.tensor_add(x1v[:, no:no + nn_], pss[i][:],
                                             xv[:, mt, no:no + nn_])
                    if not zero_bias:
                        nc.vector.tensor_add(x1v[:], x1v[:], bias_bc[:, 0, :])
                    nc.sync.dma_start(out=x1d[v * L + mt * 128: v * L + (mt + 1) * 128, :],
                                      in_=x1v[:])

        # =================== phase 2 + 3 ===================
        with tc.tile_pool(name="p23", bufs=1) as p23:
            x2 = p23.tile([128, 4, C], F32)

            with tc.tile_pool(name="p2p", bufs=1) as p2p:
                k2T = p2p.tile([128, CK, S], BF16)
                v2 = p2p.tile([128, SK, C], BF16)
                h2mT = p2p.tile([128, CK, MYQ], BF16)
                q2T = p2p.tile([128, CK, MYQ], BF16)
                o2T = p2p.tile([128, CK, MYQ], BF16)

                # --- 2a: ln2 + k/v projections, streamed per view ---
                with tc.tile_pool(name="p2aw", bufs=1) as p2aw, \
                     tc.tile_pool(name="p2ab", bufs=2) as p2ab, \
                     tc.tile_pool(name="p2as", bufs=3) as p2as, \
                     tc.tile_pool(name="ps2_sc", bufs=2, space="PSUM") as ps2_sc, \
                     tc.tile_pool(name="ps2_mb", bufs=3, space="PSUM") as ps2_mb:
                    wk_s = p2aw.tile([128, CK, C], BF16)
                    wv_s = p2aw.tile([128, CK, C], BF16)
                    for kc in range(CK):
                        nc.sync.dma_start(out=wk_s[:, kc, :], in_=wk[kc * 128:(kc + 1) * 128, :])
                        nc.sync.dma_start(out=wv_s[:, kc, :], in_=wv[kc * 128:(kc + 1) * 128, :])

                    for v in range(V):
                        x1v = p2ab.tile([128, 2, C], F32, tag="x1v")
                        for mc in range(2):
                            nc.sync.dma_start(
                                out=x1v[:, mc, :],
                                in_=x1d[v * L + mc * 128: v * L + (mc + 1) * 128, :])
                        h2 = p2ab.tile([128, 2, C], BF16, tag="h2")
                        for mc in range(2):
                            ln(p2as, x1v[:, mc, :], h2[:, mc, :], 1)
                        h2T = p2ab.tile([128, CK, 256], BF16, tag="h2T")
                        for j in range(CK):
                            transpose_cols(ps2_sc,
                                           lambda mc: h2[:, mc, j * 128:(j + 1) * 128],
                                           h2T, j, 2)
                        for kc in range(CK):
                            nc.any.tensor_copy(h2mT[:, kc, v * QS:(v + 1) * QS],
                                               h2T[:, kc, 0:QS])
                        for mo in range(CK):
                            ps = ps2_sc.tile([128, 256], F32, tag="sc")
                            for kc in range(CK):
                                nc.tensor.matmul(ps[:],
                                                 wk_s[:, kc, mo * 128:(mo + 1) * 128],
                                                 h2T[:, kc, :],
                                                 start=kc == 0, stop=kc == CK - 1)
                            nc.any.tensor_copy(k2T[:, mo, v * L:(v + 1) * L], ps[:])
                        for mt in range(2):
                            pss = [ps2_mb.tile([128, 384], F32, tag="mb", name=f"mbh{i}") for i in range(2)]
                            for kc in range(CK):
                                for i, (no, nn_) in enumerate(NHALF):
                                    nc.tensor.matmul(pss[i][:],
                                                     h2T[:, kc, mt * 128:(mt + 1) * 128],
                                                     wv_s[:, kc, no:no + nn_],
                                                     start=kc == 0, stop=kc == CK - 1)
                            for i, (no, nn_) in enumerate(NHALF):
                                nc.any.tensor_copy(v2[:, v * 2 + mt, no:no + nn_], pss[i][:])

                # --- q projection for my 512 tokens ---
                with tc.tile_pool(name="p2qw", bufs=1) as p2qw, \
                     tc.tile_pool(name="ps2q", bufs=2, space="PSUM") as ps2q:
                    wq_s = p2qw.tile([128, CK, C], BF16)
                    for kc in range(CK):
                        nc.sync.dma_start(out=wq_s[:, kc, :], in_=wq[kc * 128:(kc + 1) * 128, :])
                    for mo in range(CK):
                        ps = ps2q.tile([128, MYQ], F32)
                        for kc in range(CK):
                            nc.tensor.matmul(ps[:], wq_s[:, kc, mo * 128:(mo + 1) * 128],
                                             h2mT[:, kc, :],
                                             start=kc == 0, stop=kc == CK - 1)
                        nc.any.tensor_copy(q2T[:, mo, :], ps[:])

                # --- 2b: block-causal attention over key prefixes ---
                with tc.tile_pool(name="p2bs", bufs=3) as p2bs, \
                     tc.tile_pool(name="ps2b_sc", bufs=3, space="PSUM") as ps2b_sc, \
                     tc.tile_pool(name="ps2b_o", bufs=2, space="PSUM") as ps2b_o:
                    for hp in range(6):
                        ops = ps2b_o.tile([128, MYQ], F32, tag="o")
                        for hh in range(2):
                            h = hp * 2 + hh
                            for v in range(V):
                                kr = _kr(v)
                                nk = (kr + 511) // 512
                                nkb = kr // 128
                                qh = q2T[hh * 64:(hh + 1) * 64, hp, v * QS:(v + 1) * QS]
                                probs = p2bs.tile([64, S], BF16, tag="probs2")
                                sums = p2bs.tile([64, 4], F32, tag="sums2")
                                for ck in range(nk):
                                    kw = min(512, kr - ck * 512)
                                    sps = ps2b_sc.tile([64, 512], F32, tag="sc")
                                    nc.tensor.matmul(sps[:, :kw], qh,
                                                     k2T[hh * 64:(hh + 1) * 64, hp,
                                                         ck * 512:ck * 512 + kw],
                                                     start=True, stop=True)
                                    nc.scalar.activation(probs[:, ck * 512:ck * 512 + kw],
                                                         sps[:, :kw], AF.Exp, scale=SCALE,
                                                         accum_out=sums[:, ck:ck + 1])
                                rtot = p2bs.tile([64, 1], F32, tag="rtot")
                                nc.vector.reduce_sum(out=rtot[:], in_=sums[:, 0:nk],
                                                     axis=mybir.AxisListType.X)
                                nc.vector.reciprocal(rtot[:], rtot[:])
                                nc.vector.tensor_scalar_mul(probs[:, :kr], probs[:, :kr],
                                                            rtot[:])
                                pT = p2bs.tile([128, SK, QS], BF16, tag="pT2")
                                for g4 in range((nkb + 3) // 4):
                                    nb = min(4, nkb - g4 * 4)
                                    pTps = ps2b_sc.tile([128, 4, QS], BF16, tag="scb")
                                    for i in range(nb):
                                        kb = g4 * 4 + i
                                        nc.tensor.matmul(pTps[:, i, :],
                                                         probs[:, kb * 128:(kb + 1) * 128],
                                                         identb[0:64, 0:QS],
                                                         is_transpose=True)
                                    nc.any.tensor_copy(pT[:, g4 * 4:g4 * 4 + nb, :],
                                                       pTps[:, 0:nb, :])
                                for kb in range(nkb):
                                    nc.tensor.matmul(
                                        ops[hh * 64:(hh + 1) * 64, v * QS:(v + 1) * QS],
                                        v2[:, kb, h * 64:(h + 1) * 64], pT[:, kb, :],
                                        start=kb == 0, stop=kb == nkb - 1)
                        nc.any.tensor_copy(o2T[:, hp, :], ops[:])

                # --- 2c: cproj + residual ---
                with tc.tile_pool(name="p2cw", bufs=1) as p2cw, \
                     tc.tile_pool(name="p2cs", bufs=2) as p2cs, \
                     tc.tile_pool(name="ps2c", bufs=3, space="PSUM") as ps2c:
                    wcp_s = p2cw.tile([128, CK, C], BF16)
                    for kc in range(CK):
                        nc.sync.dma_start(out=wcp_s[:, kc, :],
                                          in_=wcp[kc * 128:(kc + 1) * 128, :])
                    x1m = p2cw.tile([128, 4, C], F32)
                    for v in range(V):
                        nc.sync.dma_start(out=x1m[(v % 2) * 64:(v % 2) * 64 + 64, v // 2, :],
                                          in_=x1d[v * L: v * L + QS, :])
                    for mt in range(4):
                        pss = [ps2c.tile([128, 384], F32, tag="mb", name=f"mbh{i}") for i in range(2)]
                        for kc in range(CK):
                            for i, (no, nn_) in enumerate(NHALF):
                                nc.tensor.matmul(pss[i][:],
                                                 o2T[:, kc, mt * 128:(mt + 1) * 128],
                                                 wcp_s[:, kc, no:no + nn_],
                                                 start=kc == 0, stop=kc == CK - 1)
                        for i, (no, nn_) in enumerate(NHALF):
                            nc.vector.tensor_add(x2[:, mt, no:no + nn_], pss[i][:],
                                                 x1m[:, mt, no:no + nn_])
                        if not zero_bias:
                            nc.vector.tensor_add(x2[:, mt, :], x2[:, mt, :],
                                                 bias_bc[:, 1, :])

            # =================== phase 3: MLP ===================
            with tc.tile_pool(name="p3w", bufs=1) as p3w, \
                 tc.tile_pool(name="p3one", bufs=1) as p3one, \
                 tc.tile_pool(name="p3s", bufs=3) as p3s, \
                 tc.tile_pool(name="ps3_sc", bufs=2, space="PSUM") as ps3_sc, \
                 tc.tile_pool(name="ps3_mb", bufs=3, space="PSUM") as ps3_mb:
                wf1_s = p3w.tile([128, CK, HID], BF16)
                for kc in range(CK):
                    nc.sync.dma_start(out=wf1_s[:, kc, :], in_=wf1[kc * 128:(kc + 1) * 128, :])
                wf2_s = p3w.tile([128, HK, C], BF16)
                for kc in range(HK):
                    nc.sync.dma_start(out=wf2_s[:, kc, :], in_=wf2[kc * 128:(kc + 1) * 128, :])

                h3 = p3one.tile([128, 4, C], BF16)
                for mt in range(4):
                    ln(p3s, x2[:, mt, :], h3[:, mt, :], 2)
                h3T = p3one.tile([128, CK, MYQ], BF16)
                for j in range(CK):
                    transpose_cols(ps3_sc, lambda mc: h3[:, mc, j * 128:(j + 1) * 128],
                                   h3T, j, 4)
                g1T = p3one.tile([128, HK, MYQ], BF16)
                for mo in range(HK):
                    ps = ps3_sc.tile([128, MYQ], F32, tag="sc")
                    for kc in range(CK):
                        nc.tensor.matmul(ps[:], wf1_s[:, kc, mo * 128:(mo + 1) * 128],
                                         h3T[:, kc, :], start=kc == 0, stop=kc == CK - 1)
                    if sim_gelu:
                        # tanh-approx gelu from sim-supported ops (sim only)
                        xg = p3s.tile([128, MYQ], F32, tag="xg")
                        if zero_bias:
                            nc.any.tensor_copy(xg[:], ps[:])
                        else:
                            nc.scalar.activation(xg[:], ps[:], AF.Identity,
                                                 bias=f1b_t[:, mo:mo + 1])
                        x2g = p3s.tile([128, MYQ], F32, tag="x2g")
                        nc.scalar.activation(x2g[:], xg[:], AF.Square)
                        nc.vector.tensor_scalar(x2g[:], x2g[:], 0.0356774081,
                                                0.7978845608, ALU.mult, ALU.add)
                        nc.vector.tensor_mul(x2g[:], x2g[:], xg[:])
                        nc.scalar.activation(x2g[:], x2g[:], AF.Tanh)
                        nc.vector.tensor_mul(x2g[:], x2g[:], xg[:])
                        nc.vector.tensor_add(x2g[:], x2g[:], xg[:])
                        nc.vector.tensor_scalar_mul(x2g[:], x2g[:], 0.5)
                        nc.any.tensor_copy(g1T[:, mo, :], x2g[:])
                    elif zero_bias:
                        nc.scalar.activation(g1T[:, mo, :], ps[:], AF.Gelu)
                    else:
                        nc.scalar.activation(g1T[:, mo, :], ps[:], AF.Gelu,
                                             bias=f1b_t[:, mo:mo + 1])
                for mt in range(4):
                    pss = [ps3_mb.tile([128, 384], F32, tag="mb", name=f"mbh{i}") for i in range(2)]
                    for kc in range(HK):
                        for i, (no, nn_) in enumerate(NHALF):
                            nc.tensor.matmul(pss[i][:],
                                             g1T[:, kc, mt * 128:(mt + 1) * 128],
                                             wf2_s[:, kc, no:no + nn_],
                                             start=kc == 0, stop=kc == HK - 1)
                    yo = p3s.tile([128, C], F32, tag="yo")
                    for i, (no, nn_) in enumerate(NHALF):
                        nc.vector.tensor_add(yo[:, no:no + nn_], pss[i][:],
                                             x2[:, mt, no:no + nn_])
                    if not zero_bias:
                        nc.vector.tensor_add(yo[:], yo[:], bias_bc[:, 2, :])
                    nc.sync.dma_start(out=out[mt * 128:(mt + 1) * 128, :], in_=yo[:])

    nc.finalize()
    return nc


_CACHE = {}


def _get_nc(ln_identity, zero_bias, sim_gelu=False):
    key = (ln_identity, zero_bias, sim_gelu)
    if key not in _CACHE:
        _CACHE[key] = _build(ln_identity, zero_bias, sim_gelu)
    return _CACHE[key]


def _prep_inputs(inputs):
    x = np.asarray(inputs["x"], np.float32)          # [B, V, L, C]
    ln_identity = all(np.all(np.asarray(inputs[f"ln{i}_g"]) == 1.0)
                      and np.all(np.asarray(inputs[f"ln{i}_b"]) == 0.0)
                      for i in (1, 2, 3))
    zero_bias = all(np.all(np.asarray(inputs[k]) == 0.0)
                    for k in ("attn_proj_b", "cproj_b", "fc1_b", "fc2_b"))

    tr = lambda k: np.ascontiguousarray(
        np.asarray(inputs[k], np.float32).T).astype(ml_dtypes.bfloat16)
    wqkv_t, wproj_t = tr("qkv_w"), tr("attn_proj_w")
    wq_t, wk_t, wv_t, wcp_t = tr("q_w"), tr("k_w"), tr("v_w"), tr("cproj_w")
    wf1_t = tr("fc1_w")
    wf2_t = tr("fc2_w")

    in_maps = []
    for c in range(NCORES):
        b, g = divmod(c, G)
        xbp = np.empty((S, C), np.float32)
        for v in range(V):
            xv = x[b, v]
            xbp[v * L: v * L + QS] = xv[g * QS:(g + 1) * QS]
            xbp[v * L + QS: v * L + QS + g * QS] = xv[0: g * QS]
            xbp[v * L + QS + g * QS: (v + 1) * L] = xv[(g + 1) * QS:]
        m = {"xb": xbp, "wqkv_t": wqkv_t, "wproj_t": wproj_t, "wq_t": wq_t,
             "wk_t": wk_t, "wv_t": wv_t, "wcproj_t": wcp_t, "wfc1_t": wf1_t,
             "wfc2_t": wf2_t}
        if not ln_identity:
            m["ln_g"] = np.stack([np.asarray(inputs[f"ln{i}_g"], np.float32)
                                  for i in (1, 2, 3)])
            m["ln_b"] = np.stack([np.asarray(inputs[f"ln{i}_b"], np.float32)
                                  for i in (1, 2, 3)])
        if not zero_bias:
            m["bias3"] = np.stack([np.asarray(inputs["attn_proj_b"], np.float32),
                                   np.asarray(inputs["cproj_b"], np.float32),
                                   np.asarray(inputs["fc2_b"], np.float32)])
            m["fc1_b"] = np.asarray(inputs["fc1_b"], np.float32)
        in_maps.append(m)
    return in_maps, ln_identity, zero_bias


def _assemble(results):
    out = np.empty((B, V, L, C), np.float32)
    for c in range(NCORES):
        b, g = divmod(c, G)
        oc = np.asarray(results[c]["out"])
        for v in range(V):
            out[b, v, g * QS:(g + 1) * QS] = oc[v * QS:(v + 1) * QS]
    return out


def kernel(**inputs):
    in_maps, ln_identity, zero_bias = _prep_inputs(inputs)
    nc = _get_nc(ln_identity, zero_bias)
    res = run_bass_kernel_spmd(nc, in_maps, core_ids=list(range(NCORES)))
    return _assemble(res.results)

